# revision 6
# baseline (speedup 1.0000x reference)
"""CenterLoss Trainium2 kernel (8-core SPMD, data-parallel over batch).

loss = mean_i( ||feat_i - centers[label_i]|| / count[label_i] )

Device algorithm (per core, batch shard of 2048 rows):
  - radix-100 class factorization: c = 100*h + l
  - count2d[h,l] = sum_i 1[h_i==h] * 1[l_i==l]   (global histogram, exact,
    bf16 one-hot matmuls accumulated in fp32 PSUM)
  - dist_i = ||feat_i - centers[label_i]||  (dma_gather of center rows,
    DVE subtract, ACT square+accumulate, ACT sqrt)
  - S2d[h,l] = sum_{i in shard} onehot_h[i,h] * onehot_l[i,l] * dist_i
  - partial = sum_{h,l} S2d / max(count2d, 1)
  - host: loss = sum_k partial_k / B    (exact: count is global, S local)
"""

from contextlib import ExitStack

import numpy as np

import concourse.bass as bass
import concourse.tile as tile
from concourse import bacc, mybir
from concourse import bass_utils
from concourse.alu_op_type import AluOpType

B, D, C = 16384, 512, 10000
NCORES = 8
BLOC = B // NCORES  # 2048 rows per core
P = 128
TLOC = BLOC // P    # 16 local batch tiles
TG = B // P         # 128 global batch tiles
R = 100             # radix (c = 100*h + l)
JCHUNK = 32         # global tiles per one-hot chunk
NCHUNK = TG // JCHUNK
DCHUNK = 4          # local tiles per dist DMA chunk
NDC = TLOC // DCHUNK

F32 = mybir.dt.float32
BF16 = mybir.dt.bfloat16
I16 = mybir.dt.int16

_CACHE: dict = {}


def build_program(reps: int = 1):
    """Build + compile the per-core Bass program (SPMD: same program on
    all 8 cores, different input data).

    reps > 1 repeats the whole body, chained through a scalar so DCE keeps
    every rep (for timing: marginal wall-clock per rep = pure device time).
    """
    nc = bacc.Bacc(
        "TRN2", target_bir_lowering=False, debug=False, enable_asserts=False
    )

    feat_d = nc.dram_tensor("feat", [BLOC, D], F32, kind="ExternalInput").ap()
    cent_d = nc.dram_tensor("centers", [C, D], F32, kind="ExternalInput").ap()
    gidx_d = nc.dram_tensor("gidx", [P, BLOC // 16], I16, kind="ExternalInput").ap()
    hq_d = nc.dram_tensor("hq", [P, TG], I16, kind="ExternalInput").ap()
    lq_d = nc.dram_tensor("lq", [P, TG], I16, kind="ExternalInput").ap()
    hloc_d = nc.dram_tensor("hloc", [P, TLOC], I16, kind="ExternalInput").ap()
    lloc_d = nc.dram_tensor("lloc", [P, TLOC], I16, kind="ExternalInput").ap()
    tok_d = nc.dram_tensor("tok", [1, 1], F32, kind="ExternalInput").ap()
    out_d = nc.dram_tensor("out", [1, 1], F32, kind="ExternalOutput").ap()

    feat_r = feat_d.rearrange("(p t) d -> p t d", p=P)

    with tile.TileContext(nc) as tc, ExitStack() as ctx:
        const = ctx.enter_context(tc.tile_pool(name="const", bufs=1))
        big = ctx.enter_context(tc.tile_pool(name="big", bufs=3))
        oh = ctx.enter_context(tc.tile_pool(name="oh", bufs=2))
        work = ctx.enter_context(tc.tile_pool(name="work", bufs=3))
        fin = ctx.enter_context(tc.tile_pool(name="fin", bufs=1))
        psum = ctx.enter_context(tc.tile_pool(name="psum", bufs=1, space="PSUM"))

        chain_prev = None
        for _rep in range(reps):
            # ---- small input loads
            hq_s = const.tile([P, TG], I16, tag="hq")
            nc.sync.dma_start(hq_s[:], hq_d[:])
            lq_s = const.tile([P, TG], I16, tag="lq")
            nc.sync.dma_start(lq_s[:], lq_d[:])
            hloc_s = const.tile([P, TLOC], I16, tag="hloc")
            nc.sync.dma_start(hloc_s[:], hloc_d[:])
            lloc_s = const.tile([P, TLOC], I16, tag="lloc")
            nc.sync.dma_start(lloc_s[:], lloc_d[:])
            gidx_s = const.tile([P, BLOC // 16], I16, tag="gidx")
            nc.sync.dma_start(gidx_s[:], gidx_d[:])
            tok_s = const.tile([1, 1], F32, tag="tok")
            nc.sync.dma_start(tok_s[:], tok_d[:])

            # ---- constants: iota[p, h, j] = h (int16); ones column (f32)
            iota_s = const.tile([P, R, JCHUNK], I16, tag="iota")
            nc.gpsimd.iota(
                iota_s[:], pattern=[[1, R], [0, JCHUNK]], base=0, channel_multiplier=0
            )
            iota16 = iota_s[:, :, 0:TLOC]
            ones_s = const.tile([R, 1], F32, tag="ones")
            nc.vector.memset(ones_s[:], 1.0)

            # ---- dist path: chunked feat DMA + chunked center-row gather,
            # sub (DVE) + square-accumulate (ACT) pipelined per chunk
            dist2_s = fin.tile([P, TLOC], F32, tag="dist2")
            gcols = (BLOC // 16) // NDC  # gidx columns per chunk
            for q in range(NDC):
                feat_c = big.tile([P, DCHUNK, D], F32, tag="feat")
                nc.sync.dma_start(
                    feat_c[:], feat_r[:, q * DCHUNK : (q + 1) * DCHUNK]
                )
                gath_c = big.tile([P, DCHUNK, D], F32, tag="gath")
                nc.gpsimd.dma_gather(
                    out_ap=gath_c[:],
                    in_ap=cent_d[:],
                    idxs_ap=gidx_s[:, q * gcols : (q + 1) * gcols],
                    num_idxs=BLOC // NDC,
                    num_idxs_reg=BLOC // NDC,
                    elem_size=D,
                    single_packet=False,
                )
                for t in range(DCHUNK):
                    diff = work.tile([P, D], F32, tag="diff")
                    nc.vector.tensor_sub(diff[:], feat_c[:, t], gath_c[:, t])
                    sq = work.tile([P, D], F32, tag="sq")
                    nc.scalar.activation(
                        sq[:],
                        diff[:],
                        mybir.ActivationFunctionType.Square,
                        accum_out=dist2_s[:, q * DCHUNK + t : q * DCHUNK + t + 1],
                    )
            dist_s = fin.tile([P, TLOC], F32, tag="dist")
            nc.scalar.activation(
                dist_s[:], dist2_s[:], mybir.ActivationFunctionType.Sqrt
            )
            dist_bf = fin.tile([P, TLOC], BF16, tag="dist_bf")
            nc.vector.tensor_copy(dist_bf[:], dist_s[:])

            # ---- global one-hot + count matmuls, chunked over global tiles
            psum_cnt = psum.tile([R, R], F32, tag="psum_cnt")
            for c in range(NCHUNK):
                hq_b = (
                    hq_s[:, c * JCHUNK : (c + 1) * JCHUNK]
                    .unsqueeze(1)
                    .broadcast_to([P, R, JCHUNK])
                )
                lq_b = (
                    lq_s[:, c * JCHUNK : (c + 1) * JCHUNK]
                    .unsqueeze(1)
                    .broadcast_to([P, R, JCHUNK])
                )
                a_c = oh.tile([P, R, JCHUNK], BF16, tag="a_c")
                nc.vector.tensor_tensor(a_c[:], hq_b, iota_s[:], AluOpType.is_equal)
                b_c = oh.tile([P, R, JCHUNK], BF16, tag="b_c")
                nc.vector.tensor_tensor(b_c[:], lq_b, iota_s[:], AluOpType.is_equal)
                for j in range(JCHUNK):
                    nc.tensor.matmul(
                        psum_cnt[:],
                        a_c[:, :, j],
                        b_c[:, :, j],
                        start=(c == 0 and j == 0),
                        stop=(c == NCHUNK - 1 and j == JCHUNK - 1),
                    )
            # 1 / max(count, 1): ready as soon as count matmuls close
            cnt_sb = fin.tile([R, R], F32, tag="cnt_sb")
            nc.vector.tensor_scalar_max(cnt_sb[:], psum_cnt[:], 1.0)
            recip_sb = fin.tile([R, R], F32, tag="recip_sb")
            nc.vector.reciprocal(recip_sb[:], cnt_sb[:])

            # ---- local one-hots, dist-scaled, and S matmuls
            hloc_b = hloc_s[:].unsqueeze(1).broadcast_to([P, R, TLOC])
            lloc_b = lloc_s[:].unsqueeze(1).broadcast_to([P, R, TLOC])
            dist_b = dist_bf[:].unsqueeze(1).broadcast_to([P, R, TLOC])
            a_loc = fin.tile([P, R, TLOC], BF16, tag="a_loc")
            nc.vector.tensor_tensor(a_loc[:], hloc_b, iota16, AluOpType.is_equal)
            b_loc = fin.tile([P, R, TLOC], BF16, tag="b_loc")
            nc.vector.tensor_tensor(b_loc[:], lloc_b, iota16, AluOpType.is_equal)
            bp_loc = fin.tile([P, R, TLOC], BF16, tag="bp_loc")
            nc.vector.tensor_tensor(bp_loc[:], b_loc[:], dist_b, AluOpType.mult)

            psum_s = psum.tile([R, R], F32, tag="psum_s")
            for t in range(TLOC):
                nc.tensor.matmul(
                    psum_s[:],
                    a_loc[:, :, t],
                    bp_loc[:, :, t],
                    start=(t == 0),
                    stop=(t == TLOC - 1),
                )

            # ---- partial = sum_{h,l} S / max(count, 1)
            ratio = fin.tile([R, R], F32, tag="ratio")
            nc.vector.tensor_mul(ratio[:], psum_s[:], recip_sb[:])
            rowsum = fin.tile([R, 1], F32, tag="rowsum")
            nc.vector.tensor_reduce(
                rowsum[:], ratio[:], axis=mybir.AxisListType.X, op=AluOpType.add
            )
            # partition reduce via PE: total = rowsum^T @ ones
            psum_tot = psum.tile([1, 1], F32, tag="psum_tot")
            nc.tensor.matmul(psum_tot[:], rowsum[:], ones_s[:], start=True, stop=True)

            # chained output: out = partial + 0 * prev  (keeps reps live
            # under DCE when reps > 1; per-rep work still pipelines)
            out_s = fin.tile([1, 1], F32, tag=f"out_s{_rep}")
            prev = tok_s if _rep == 0 else chain_prev
            nc.vector.scalar_tensor_tensor(
                out=out_s[:],
                in0=prev[:],
                scalar=0.0,
                in1=psum_tot[:],
                op0=AluOpType.mult,
                op1=AluOpType.add,
            )
            chain_prev = out_s
        nc.sync.dma_start(out_d[:], chain_prev[:])

    nc.compile()
    return nc


def make_in_maps(feat, label, centers, tok=0.0):
    """Shard + lay out full inputs into the 8 per-core input maps."""
    feat = np.ascontiguousarray(np.asarray(feat, dtype=np.float32))
    label = np.asarray(label, dtype=np.int32)
    centers = np.ascontiguousarray(np.asarray(centers, dtype=np.float32))

    h_all = (label // R).astype(np.int16)
    l_all = (label % R).astype(np.int16)
    hq = h_all.reshape(P, TG)  # sample i = p*128 + j at [p, j]
    lq = l_all.reshape(P, TG)
    g = np.arange(BLOC)
    perm = (g % P) * TLOC + (g // P)  # gather order -> local row index
    tok_arr = np.full((1, 1), tok, dtype=np.float32)

    in_maps = []
    for k in range(NCORES):
        lab_k = label[k * BLOC : (k + 1) * BLOC]
        gvals = lab_k[perm].astype(np.int16)  # idx list in gather order
        gidx16 = np.ascontiguousarray(gvals.reshape(BLOC // 16, 16).T)  # [16, 128]
        gidx = np.ascontiguousarray(np.tile(gidx16, (P // 16, 1)))
        in_maps.append(
            {
                "feat": feat[k * BLOC : (k + 1) * BLOC],
                "centers": centers,
                "gidx": gidx,
                "hq": hq,
                "lq": lq,
                "hloc": np.ascontiguousarray(
                    (lab_k // R).astype(np.int16).reshape(P, TLOC)
                ),
                "lloc": np.ascontiguousarray(
                    (lab_k % R).astype(np.int16).reshape(P, TLOC)
                ),
                "tok": tok_arr,
            }
        )
    return in_maps


def get_program():
    if "nc" not in _CACHE:
        _CACHE["nc"] = build_program()
    return _CACHE["nc"]


def kernel(feat, label, centers):
    nc = get_program()
    in_maps = make_in_maps(feat, label, centers)
    res = bass_utils.run_bass_kernel_spmd(nc, in_maps, core_ids=list(range(NCORES)))
    total = sum(float(res.results[k]["out"][0, 0]) for k in range(NCORES))
    return np.asarray(total / B, dtype=np.float32)
